# revision 1
# baseline (speedup 1.0000x reference)
"""Trainium2 Bass kernel for nn_AttentionHead (B=4, T=4096, D=1024, H=64).

Sharding: 8 cores; core i handles (batch b = i//2, T-half = i%2): computes
attention output for its 2048 queries. K/V are computed per-core over the
full 4096 keys (weights tiny/replicated; key order is permutation-invariant
under softmax, so own-half-first ordering per core is fine).

Per-core dataflow (big matmuls in float32r = full-rate fp32 on the PE;
walrus requires fp32r operands to be produced *rounded*, so every matmul
input comes from a DVE copy or ACT activation with fp32r output dtype):
  - x inputs are typed float32r so the 256 PE transposes to x^T run in
    transpose mode at 1.5 cyc/row (walrus accepts external-input f32r);
    the PE pass itself rounds x to f32r precision.
  - Projections use host-concatenated stationary weights so one M=128
    pass computes two heads at once (M does not affect matmul time):
    first T-half runs [Wk|Wq] (k -> kTp partitions 0:64, q -> 64:128,
    then DMA-duplicated down), second T-half runs [Wv|Wk] so k lands
    directly at partitions 64:128 of kTp. relu+bias on ACT writes the
    persistent fp32r tiles in place; scores then row-pack two
    64-contraction matmuls (row groups 0/64) per PSUM tile.
  - v_T PE-transposed to V natural [t,64]; column 64 = ones so attn@V also
    accumulates the softmax denominator.
  - scores s_T[k,q]: two k-tiles row-packed (contraction=64, row groups
    0/64) into one PSUM [128,1024] tile; exp on ACT with scale=1/8 (no max
    subtraction: scores are O(1) by construction). Two q-blocks are
    interleaved per k-pair so PE work hides the ACT exp chain.
  - attn@V: V'[128,65] stationary x exp[128,512] accumulated over 32
    k-tiles into PSUM [65,512]; row 64 = denominator. PE-transpose back,
    reciprocal*scale on DVE, DMA out.

Tensors are split at group granularity (kTp/Vg/qTb) so the Tile scheduler
can overlap the projection stage with attention as dependencies resolve.
"""

import os
import numpy as np

B, T, D, H = 4, 4096, 1024, 64
P = 128
NB = 512            # free-dim block size
TQ = T // 2         # queries per core
NCORES = 8

_cache = {}


def _build(use_f32r=True):
    import concourse.bass as bass
    import concourse.tile as tile
    from concourse import bacc, mybir
    from concourse.masks import make_identity

    f32 = mybir.dt.float32
    f32r = mybir.dt.float32r
    AF = mybir.ActivationFunctionType

    mmdt = f32r if use_f32r else f32

    nc = bacc.Bacc("TRN2", target_bir_lowering=False, debug=False)

    xa = nc.dram_tensor("xa", [TQ, D], mmdt, kind="ExternalInput").ap()
    xb = nc.dram_tensor("xb", [TQ, D], mmdt, kind="ExternalInput").ap()
    wkq = nc.dram_tensor("wkq", [D, P], f32, kind="ExternalInput").ap()
    wvk = nc.dram_tensor("wvk", [D, P], f32, kind="ExternalInput").ap()
    wvt = nc.dram_tensor("wvt", [D, H], f32, kind="ExternalInput").ap()
    bkq = nc.dram_tensor("bkq", [P, 1], f32, kind="ExternalInput").ap()
    bvk = nc.dram_tensor("bvk", [P, 1], f32, kind="ExternalInput").ap()
    bv = nc.dram_tensor("bv", [H, 1], f32, kind="ExternalInput").ap()
    out = nc.dram_tensor("o", [TQ, H], f32, kind="ExternalOutput").ap()

    NG = T // NB          # 8 K/V t-groups of 512
    NGH = NG // 2         # 4 groups per T-half
    NQB = TQ // NB        # 4 q-blocks of 512
    NKP = T // P // 2     # 16 k-tile pairs
    NC = D // P           # 8 d-chunks
    NJ = NB // P          # 4 t-subtiles per group

    with tile.TileContext(nc) as tc:
        with (
            tc.tile_pool(name="const", bufs=1) as constp,
            tc.tile_pool(name="persist", bufs=1) as persist,
            tc.tile_pool(name="xg", bufs=6) as xgp,
            tc.tile_pool(name="xt", bufs=8) as xtp,
            tc.tile_pool(name="kvsb", bufs=2) as kvp,
            tc.tile_pool(name="big_ps", bufs=2, space="PSUM") as bigp,
            tc.tile_pool(name="proj_ps", bufs=2, space="PSUM") as proj_psp,
            tc.tile_pool(name="o_ps", bufs=2, space="PSUM") as o_psp,
            tc.tile_pool(name="esb", bufs=4) as esbp,
            tc.tile_pool(name="osb", bufs=2) as osbp,
            tc.tile_pool(name="outp", bufs=2) as outp,
        ):
            ident = constp.tile([P, P], f32)
            make_identity(nc, ident)
            ident_r = constp.tile([P, P], mmdt)
            nc.vector.tensor_copy(ident_r, ident)
            wkq_ld = constp.tile([P, NC, P], f32)
            wvk_ld = constp.tile([P, NC, P], f32)
            wv_ld = constp.tile([P, NC, H], f32)
            wkq_sb = constp.tile([P, NC, P], mmdt)
            wvk_sb = constp.tile([P, NC, P], mmdt)
            wv_sb = constp.tile([P, NC, H], mmdt)
            bkq_sb = constp.tile([P, 1], f32)
            bvk_sb = constp.tile([P, 1], f32)
            bv_sb = constp.tile([H, 1], f32)

            def load_consts():
                # emitted after the first x-block DMA so the transposes (the
                # first PE work) aren't stuck behind the weight loads; DVE
                # copies round fp32 -> fp32r (walrus requires rounded inputs)
                nc.sync.dma_start(wkq_ld, wkq.rearrange("(c p) h -> p c h", p=P))
                nc.sync.dma_start(wvk_ld, wvk.rearrange("(c p) h -> p c h", p=P))
                nc.sync.dma_start(wv_ld, wvt.rearrange("(c p) h -> p c h", p=P))
                nc.vector.tensor_copy(wkq_sb, wkq_ld)
                nc.vector.tensor_copy(wvk_sb, wvk_ld)
                nc.vector.tensor_copy(wv_sb, wv_ld)
                nc.sync.dma_start(bkq_sb, bkq)
                nc.sync.dma_start(bvk_sb, bvk)
                nc.sync.dma_start(bv_sb, bv)

            # persistent attention operands, split per group for overlap
            kTp = [persist.tile([P, NJ, P], mmdt, name=f"kTp{j}") for j in range(NGH)]
            qTb = [persist.tile([P, NB], mmdt, name=f"qTb{j}") for j in range(NQB)]
            Vg = [persist.tile([P, NJ, H + 1], mmdt, name=f"Vg{g}") for g in range(NG)]
            onesc = constp.tile([P, NJ, 1], f32)
            nc.gpsimd.memset(onesc, 1.0)
            for g in range(NG):
                nc.vector.tensor_copy(Vg[g][:, :, H : H + 1], onesc)

            # ---------------- projections ----------------
            def do_group(g, after_dma=None):
                half2 = g >= NGH          # second T-half (keys 2048..4095)
                src = xb if half2 else xa
                j = g % NGH
                r0 = j * NB
                # two half-loads so transposes start after the first lands
                xga = xgp.tile([P, NJ // 2, D], mmdt, tag="xg")
                xgb = xgp.tile([P, NJ // 2, D], mmdt, tag="xg")
                nc.sync.dma_start(
                    xga,
                    src[r0 : r0 + NB // 2, :].rearrange("(j p) d -> p j d", p=P),
                )
                nc.sync.dma_start(
                    xgb,
                    src[r0 + NB // 2 : r0 + NB, :].rearrange("(j p) d -> p j d", p=P),
                )
                if after_dma is not None:
                    after_dma()

                def xg(jj):
                    return (xga if jj < NJ // 2 else xgb)[:, jj % (NJ // 2), :]
                # transpose to x^T: c-pairs staged through one [128,1024] bank-pair
                xts = []
                for cp in range(NC // 2):
                    pt = bigp.tile([P, 2 * NB], mmdt, tag="big")
                    for ci in range(2):
                        c = 2 * cp + ci
                        for jj in range(NJ):
                            nc.tensor.transpose(
                                pt[:, ci * NB + jj * P : ci * NB + (jj + 1) * P],
                                xg(jj)[:, c * P : (c + 1) * P],
                                ident_r,
                            )
                    xt = xtp.tile([P, 2, NB], mmdt)
                    nc.vector.tensor_copy(xt, pt.rearrange("p (c n) -> p c n", c=2))
                    xts.append(xt)

                # Projections with concatenated stationary weights: one
                # M=128 pass computes two heads at once (M does not affect
                # matmul time). First half: [Wk|Wq] -> k at partitions 0:64
                # (kTp half A) and q at 64:128 (row-packed scores' B operand).
                # Second half: [Wv|Wk] -> k lands directly at partitions
                # 64:128 of kTp (no partition-shift DMA needed).
                w2 = wvk_sb if half2 else wkq_sb
                kq_ps = proj_psp.tile([P, NB], f32, tag="proj", name="kq_ps")
                for c in range(NC):
                    nc.tensor.matmul(
                        kq_ps,
                        w2[:, c, :],
                        xts[c // 2][:, c % 2, :],
                        start=(c == 0),
                        stop=(c == NC - 1),
                    )
                b2 = bvk_sb if half2 else bkq_sb
                if not half2:
                    # k rows 0:64 -> kTp half A; q rows 64:128 -> qTb
                    nc.scalar.activation(
                        kTp[j][0:H, :, :].rearrange("h j t -> h (j t)"),
                        kq_ps[0:H, :], AF.Relu, bias=b2[0:H, 0:1],
                    )
                    nc.scalar.activation(
                        qTb[j][H:P, :], kq_ps[H:P, :], AF.Relu,
                        bias=b2[H:P, 0:1],
                    )
                    nc.sync.dma_start(qTb[j][0:H, :], qTb[j][H:P, :])
                    # V in its own pass
                    v_ps = proj_psp.tile([H, NB], f32, tag="proj", name="v_ps")
                    for c in range(NC):
                        nc.tensor.matmul(
                            v_ps,
                            wv_sb[:, c, :],
                            xts[c // 2][:, c % 2, :],
                            start=(c == 0),
                            stop=(c == NC - 1),
                        )
                    v_sb = kvp.tile([H, NB], mmdt, tag="kvsb", name="v_sb")
                    nc.scalar.activation(v_sb, v_ps, AF.Relu, bias=bv_sb[:, 0:1])
                else:
                    # v rows 0:64; k rows 64:128 -> kTp half B directly
                    v_sb = kvp.tile([H, NB], mmdt, tag="kvsb", name="v_sb")
                    nc.scalar.activation(
                        v_sb, kq_ps[0:H, :], AF.Relu, bias=b2[0:H, 0:1]
                    )
                    nc.scalar.activation(
                        kTp[j][H:P, :, :].rearrange("h j t -> h (j t)"),
                        kq_ps[H:P, :], AF.Relu, bias=b2[H:P, 0:1],
                    )
                vt_ps = proj_psp.tile([P, NJ, H], mmdt, tag="proj", name="vt_ps")
                for jj in range(NJ):
                    nc.tensor.transpose(
                        vt_ps[:, jj, :],
                        v_sb[:, jj * P : (jj + 1) * P],
                        ident_r[0:H, 0:H],
                    )
                nc.vector.tensor_copy(Vg[g][:, :, 0:H], vt_ps)

            for j in range(NGH):
                do_group(j, after_dma=load_consts if j == 0 else None)
                do_group(j + NGH)

            # ---------------- attention ----------------
            scale = float(1.0 / np.sqrt(H))
            for qbp in range(NQB // 2):
                qbs = (2 * qbp, 2 * qbp + 1)
                o_ps = {
                    qb: o_psp.tile([H + 1, NB], f32, name=f"o_ps{qb}", tag="o_ps")
                    for qb in qbs
                }
                for p in range(NKP):
                    jg, i = p // NJ, p % NJ
                    e2 = {}
                    for qb in qbs:
                        s2 = bigp.tile([P, 2 * NB], f32, tag="big")
                        nc.tensor.matmul(
                            s2[:, 0:NB],
                            kTp[jg][0:H, i, :],
                            qTb[qb][0:H, :],
                            start=True,
                            stop=True,
                            tile_position=(0, 0),
                        )
                        nc.tensor.matmul(
                            s2[:, NB : 2 * NB],
                            kTp[jg][H:P, i, :],
                            qTb[qb][H:P, :],
                            start=True,
                            stop=True,
                            tile_position=(H, 0),
                        )
                        e = esbp.tile([P, 2 * NB], mmdt)
                        nc.scalar.activation(e, s2, AF.Exp, scale=scale)
                        e2[qb] = e
                    for qb in qbs:
                        nc.tensor.matmul(
                            o_ps[qb],
                            Vg[jg][:, i, :],
                            e2[qb][:, 0:NB],
                            start=(p == 0),
                            stop=False,
                        )
                        nc.tensor.matmul(
                            o_ps[qb],
                            Vg[NGH + jg][:, i, :],
                            e2[qb][:, NB : 2 * NB],
                            start=False,
                            stop=(p == NKP - 1),
                        )
                # normalize and store
                for qb in qbs:
                    o_sb = osbp.tile([H + 1, NB], f32)
                    nc.vector.tensor_copy(o_sb, o_ps[qb])
                    o4 = outp.tile([P, NJ, H], f32)
                    for jj in range(NJ):
                        ot = bigp.tile([P, H + 1], f32, tag="big")
                        nc.tensor.transpose(
                            ot,
                            o_sb[:, jj * P : (jj + 1) * P],
                            ident[0 : H + 1, 0 : H + 1],
                        )
                        recip = osbp.tile([P, 1], f32, tag="recip")
                        nc.vector.reciprocal(recip, ot[:, H : H + 1])
                        nc.vector.tensor_scalar_mul(o4[:, jj, :], ot[:, 0:H], recip)
                    q0 = qb * NB
                    nc.sync.dma_start(
                        out[q0 : q0 + NB, :].rearrange("(j p) h -> p j h", p=P), o4
                    )

    nc.compile()
    return nc


def _get_nc():
    if "nc" not in _cache:
        _cache["nc"] = _build(use_f32r=os.environ.get("K_NO_F32R", "") != "1")
    return _cache["nc"]


def _prep_inputs(x, Wk, bk, Wq, bq, Wv, bv):
    x = np.asarray(x, np.float32)
    wqt = np.asarray(Wq, np.float32).T
    wkt = np.asarray(Wk, np.float32).T
    wvt = np.ascontiguousarray(np.asarray(Wv, np.float32).T)
    wkq = np.ascontiguousarray(np.concatenate([wkt, wqt], axis=1))
    wvk = np.ascontiguousarray(np.concatenate([wvt, wkt], axis=1))
    bqc = np.asarray(bq, np.float32).reshape(H, 1)
    bkc = np.asarray(bk, np.float32).reshape(H, 1)
    bvc = np.asarray(bv, np.float32).reshape(H, 1)
    bkq = np.concatenate([bkc, bqc], axis=0)
    bvk = np.concatenate([bvc, bkc], axis=0)
    in_maps = []
    for i in range(NCORES):
        b, h = i // 2, i % 2
        xa = np.ascontiguousarray(x[b, h * TQ : (h + 1) * TQ])
        xbo = np.ascontiguousarray(x[b, (1 - h) * TQ : (2 - h) * TQ])
        in_maps.append(
            dict(xa=xa, xb=xbo, wkq=wkq, wvk=wvk, wvt=wvt,
                 bkq=bkq, bvk=bvk, bv=bvc)
        )
    return in_maps


def run(inputs, trace=False):
    from concourse.bass_utils import run_bass_kernel_spmd

    if not trace:
        # NTFF profiling is unavailable in this environment; make sure an
        # ambient BASS_TRACE can't divert the execute path.
        os.environ["BASS_NEVER_TRACE"] = "1"
    nc = _get_nc()
    in_maps = _prep_inputs(**inputs)
    res = run_bass_kernel_spmd(nc, in_maps, list(range(NCORES)), trace=trace)
    full = np.empty((B, T, H), np.float32)
    for i in range(NCORES):
        b, h = i // 2, i % 2
        full[b, h * TQ : (h + 1) * TQ] = res.results[i]["o"]
    return full, res


def kernel(**inputs):
    out, _ = run(inputs, trace=False)
    return out



# revision 16
# speedup vs baseline: 1.3041x; 1.3041x over previous
"""Trainium2 Bass kernel for nn_AttentionHead (B=4, T=4096, D=1024, H=64).

Sharding: 8 cores; core i handles (batch b = i//2, T-half = i%2): computes
attention output for its 2048 queries over all 4096 keys. K/V are computed
per-core over the full 4096 keys (weights tiny/replicated; key order is
permutation-invariant under softmax, so own-half-first ordering is fine).

Precision/engine plan (validated in numpy: max rel err ~5e-3 vs 2e-2 gate):
  - Host pre-transposes x to x^T [D, T] and casts to bf16, so NO on-device
    transposes of x are needed (kills 20us of PE transposes + 34us of DVE
    copies from the f32r baseline).
  - Projections run in bf16 (full PE rate): pass [Wk|Wq] over the own
    T-half, [Wk|Wv] over the other half, and [Wk|Wv] again over the own
    half (k half ignored) for V. PSUM rows 0:64 = k, 64:128 = q/v.
  - relu+bias+fp8 conversion happens on DVE (tensor_scalar add+max), NOT
    ACT: ACT is reserved exclusively for the exp chain (the true bottleneck
    at ~55us payload/core).
  - Scores use fp8e4 DoubleRow matmuls (0.5 cyc/row): lhsT k8 [64,2,128]
    with the j=1 contraction slot zero-padded (h=64 only), rhs q8
    [64,2,512].  Out [128 keys, 512 q] fp32 PSUM = 1 bank.
  - exp on ACT reads 3-bank PSUM score regions [128,3,512] in one wide
    instruction (amortizes the fixed PSUM/SBUF access cost) and writes
    fp8 e8 slots laid out so attn@V can consume them directly.
  - attn@V uses 256-deep DoubleRow contraction: lhsT V8 [128,2,65] (keys
    (kg*256 + j*128 + p), col 64 = ones so the same matmul accumulates the
    softmax denominator), rhs e8 [128,2,512].  16 chained matmuls per
    q-block accumulate into o_ps [65,512] (1 bank).
  - Output: DVE copy, PE transpose (f32), DVE reciprocal*scale, DMA out.

PSUM banks: proj/vt shared pool 1 + scores 2x3 + o_ps 1 = 8.
"""

import os
import numpy as np

B, T, D, H = 4, 4096, 1024, 64
P = 128
TQ = T // 2         # queries per core
GW = 512            # projection t-group width
NG = T // GW        # 8 t-groups (4 own + 4 other)
NC = D // P         # 8 contraction chunks
NKT = T // P        # 32 key tiles of 128
NKG = NKT // 2      # 16 key groups of 256
NQB = TQ // GW      # 4 query blocks of 512
EXPW = 3            # scores region width (PSUM banks per exp instruction)
NCORES = 8

_cache = {}


def _build():
    import concourse.bass as bass
    import concourse.tile as tile
    from concourse import bacc, mybir
    from concourse.masks import make_identity

    f32 = mybir.dt.float32
    bf16 = mybir.dt.bfloat16
    fp8 = mybir.dt.float8e4
    AF = mybir.ActivationFunctionType
    ALU = mybir.AluOpType
    DR = mybir.MatmulPerfMode.DoubleRow

    nc = bacc.Bacc("TRN2", target_bir_lowering=False, debug=False)

    xat = nc.dram_tensor("xat", [D, TQ], bf16, kind="ExternalInput").ap()
    xbt = nc.dram_tensor("xbt", [D, TQ], bf16, kind="ExternalInput").ap()
    wkq = nc.dram_tensor("wkq", [D, P], bf16, kind="ExternalInput").ap()
    wkv = nc.dram_tensor("wkv", [D, P], bf16, kind="ExternalInput").ap()
    bkq = nc.dram_tensor("bkq", [P, 1], f32, kind="ExternalInput").ap()
    bkv = nc.dram_tensor("bkv", [P, 1], f32, kind="ExternalInput").ap()
    out = nc.dram_tensor("o", [TQ, H], f32, kind="ExternalOutput").ap()

    scale = float(1.0 / np.sqrt(H))
    xat_r = xat.rearrange("(c p) t -> p c t", p=P)
    xbt_r = xbt.rearrange("(c p) t -> p c t", p=P)

    with tile.TileContext(nc) as tc:
        with (
            tc.tile_pool(name="const", bufs=1) as constp,
            tc.tile_pool(name="persist", bufs=1) as persist,
            tc.tile_pool(name="xg", bufs=4) as xgp,
            tc.tile_pool(name="v8", bufs=2) as v8p,
            tc.tile_pool(name="e8", bufs=2) as e8p,
            tc.tile_pool(name="osb", bufs=2) as osbp,
            tc.tile_pool(name="of", bufs=2) as ofp,
            tc.tile_pool(name="pv_ps", bufs=1, space="PSUM") as pvp,
            tc.tile_pool(name="sc_ps", bufs=2, space="PSUM") as scop,
            tc.tile_pool(name="o_ps", bufs=1, space="PSUM") as opsp,
        ):
            # ---- constants ----
            wkq_sb = constp.tile([P, NC, P], bf16)
            wkv_sb = constp.tile([P, NC, P], bf16)
            bkq_sb = constp.tile([P, 1], f32)
            bkv_sb = constp.tile([P, 1], f32)
            identf = constp.tile([P, P], f32)
            identb = constp.tile([P, P], bf16)

            # ---- persistent attention operands ----
            # DoubleRow operands must span the full 128 partitions (walrus
            # rejects 64-partition fp8 Ldweights), so the h=64 contraction is
            # zero-padded to (128 partitions x 2 k-tiles): only [0:64, kt, 0, :]
            # is live in k8, the rest is zeroed once at startup.
            k8 = persist.tile([P, NKT, 2, P], fp8, name="k8")
            q8 = persist.tile([P, 2, TQ], fp8, name="q8")
            # V8[p, kg, j, 0:64] = v of key kg*256 + j*128 + p; cols 64/65 =
            # 1.0 so the same matmul accumulates the softmax denominator (row
            # 64).  Padded to the full 128-wide stationary with zeros: walrus
            # only accepts fp8 DoubleRow Ldweights at M=128 with the k-tile
            # pair adjacent in memory (j stride == 128).
            V8 = persist.tile([P, NKG, 2, P], fp8, name="V8")

            nc.sync.dma_start(wkq_sb, wkq.rearrange("(c p) m -> p c m", p=P))
            nc.sync.dma_start(wkv_sb, wkv.rearrange("(c p) m -> p c m", p=P))
            nc.sync.dma_start(bkq_sb, bkq)
            nc.sync.dma_start(bkv_sb, bkv)
            make_identity(nc, identf)
            nc.vector.tensor_copy(identb, identf)
            # zero the padded contraction slots; uint32 bitcast views quarter
            # the element count (memset cost is per element, not per byte)
            u32 = mybir.dt.uint32
            nc.gpsimd.memset(k8[:, :, 1, :].bitcast(u32), 0)
            nc.gpsimd.memset(k8[H:P, :, 0, :].bitcast(u32), 0)
            nc.gpsimd.memset(q8[:, 1, :].bitcast(u32), 0)
            nc.gpsimd.memset(q8[H:P, 0, :].bitcast(u32), 0)
            nc.gpsimd.memset(V8.bitcast(u32), 0)
            nc.gpsimd.memset(V8[:, :, :, 64:66], 1.0)

            # ---- projections ----
            def do_group(g):
                own = g < NG // 2          # own T-half: keys 0:2048 + queries
                j = g % (NG // 2)
                src = xat_r if own else xbt_r
                t0 = j * GW
                xg = xgp.tile([P, NC, GW], bf16, tag="xg")
                nc.sync.dma_start(xg[:, 0 : NC // 2, :], src[:, 0 : NC // 2, t0 : t0 + GW])
                nc.sync.dma_start(xg[:, NC // 2 : NC, :], src[:, NC // 2 : NC, t0 : t0 + GW])

                kt0 = j * (GW // P) if own else NKT // 2 + j * (GW // P)

                def proj(w_sb):
                    ps = pvp.tile([P, GW], f32, tag="pv", name="proj_ps")
                    for c in range(NC):
                        nc.tensor.matmul(
                            ps, w_sb[:, c, :], xg[:, c, :],
                            start=(c == 0), stop=(c == NC - 1),
                        )
                    return ps

                def relu_k(ps):
                    nc.vector.tensor_scalar(
                        k8[0:H, kt0 : kt0 + GW // P, 0, :],
                        ps[0:H, :].rearrange("p (a b) -> p a b", b=P),
                        bkq_sb[0:H, 0:1], 0.0, ALU.add, ALU.max,
                    )

                def relu_v(ps):
                    # bf16 staging: walrus requires stride-2 outputs for fp8
                    # PE transposes, so transpose in bf16 and convert to fp8
                    # during the Pool copy into V8.
                    v8g = v8p.tile([H, GW], bf16, tag="v8")
                    nc.vector.tensor_scalar(
                        v8g, ps[H:P, :], bkv_sb[H:P, 0:1], 0.0, ALU.add, ALU.max,
                    )
                    vt = pvp.tile([P, GW // P, 68], bf16, tag="pv", name="vt_ps")
                    for i in range(GW // P):
                        nc.tensor.transpose(
                            vt[:, i, 0:H],
                            v8g[:, i * P : (i + 1) * P],
                            identb[0:H, 0:H],
                        )
                    for i in range(GW // P):
                        kt = kt0 + i
                        nc.vector.tensor_copy(
                            V8[:, kt // 2, kt % 2, 0:H], vt[:, i, 0:H]
                        )

                if own:
                    ps = proj(wkq_sb)
                    relu_k(ps)
                    nc.vector.tensor_scalar(
                        q8[0:H, 0, t0 : t0 + GW],
                        ps[H:P, :], bkq_sb[H:P, 0:1], 0.0, ALU.add, ALU.max,
                    )
                    relu_v(proj(wkv_sb))
                else:
                    ps = proj(wkv_sb)
                    relu_k(ps)
                    relu_v(ps)

            # interleave own/other so both k-halves stream in early
            for j in range(NG // 2):
                do_group(j)
                do_group(j + NG // 2)

            # ---- attention ----
            nregions = (NKT + EXPW - 1) // EXPW
            for qb in range(NQB):
                q0 = qb * GW
                e8 = e8p.tile([P, NKT, GW], fp8, tag="e8")
                for r in range(nregions):
                    ktr = r * EXPW
                    w = min(EXPW, NKT - ktr)
                    sc = scop.tile([P, EXPW, GW], f32, tag="sc")
                    for i in range(w):
                        nc.tensor.matmul(
                            sc[:, i, :],
                            k8[:, ktr + i, :, :],
                            q8[:, :, q0 : q0 + GW],
                            start=True, stop=True, perf_mode=DR,
                        )
                    nc.scalar.activation(
                        e8[:, ktr : ktr + w, :], sc[:, 0:w, :], AF.Exp, scale=scale
                    )
                o_ps = opsp.tile([P, GW], f32, name="o_ps")
                for kg in range(NKG):
                    nc.tensor.matmul(
                        o_ps,
                        V8[:, kg, :, :],
                        e8[:, 2 * kg : 2 * kg + 2, :],
                        start=(kg == 0), stop=(kg == NKG - 1), perf_mode=DR,
                    )
                # normalize + store
                o_sb = osbp.tile([H + 2, GW], f32, tag="osb")
                nc.vector.tensor_copy(o_sb, o_ps[0 : H + 2, :])
                oT = pvp.tile([P, GW // P, 68], f32, tag="pv", name="oT_ps")
                oF = ofp.tile([P, GW // P, H], f32, tag="of")
                for i in range(GW // P):
                    nc.tensor.transpose(
                        oT[:, i, 0 : H + 2],
                        o_sb[:, i * P : (i + 1) * P],
                        identf[0 : H + 2, 0 : H + 2],
                    )
                    recip = osbp.tile([P, 1], f32, tag="recip")
                    nc.vector.reciprocal(recip, oT[:, i, H : H + 1])
                    nc.vector.tensor_scalar_mul(oF[:, i, :], oT[:, i, 0:H], recip)
                nc.sync.dma_start(
                    out[q0 : q0 + GW, :].rearrange("(i p) h -> p i h", p=P), oF
                )

    nc.compile()
    return nc


def _get_nc():
    if "nc" not in _cache:
        _cache["nc"] = _build()
    return _cache["nc"]


def _prep_inputs(x, Wk, bk, Wq, bq, Wv, bv):
    import ml_dtypes

    bf16 = ml_dtypes.bfloat16
    x = np.asarray(x, np.float32)
    wkq = np.concatenate(
        [np.asarray(Wk, np.float32).T, np.asarray(Wq, np.float32).T], axis=1
    ).astype(bf16)
    wkv = np.concatenate(
        [np.asarray(Wk, np.float32).T, np.asarray(Wv, np.float32).T], axis=1
    ).astype(bf16)
    bkq = np.concatenate(
        [np.asarray(bk, np.float32), np.asarray(bq, np.float32)]
    ).reshape(P, 1)
    bkv = np.concatenate(
        [np.asarray(bk, np.float32), np.asarray(bv, np.float32)]
    ).reshape(P, 1)
    in_maps = []
    for i in range(NCORES):
        b, h = i // 2, i % 2
        xat = np.ascontiguousarray(x[b, h * TQ : (h + 1) * TQ].T).astype(bf16)
        xbt = np.ascontiguousarray(x[b, (1 - h) * TQ : (2 - h) * TQ].T).astype(bf16)
        in_maps.append(dict(xat=xat, xbt=xbt, wkq=wkq, wkv=wkv, bkq=bkq, bkv=bkv))
    return in_maps


def run(inputs, trace=False):
    from concourse.bass_utils import run_bass_kernel_spmd

    if not trace:
        # NTFF profiling is unavailable in this environment; make sure an
        # ambient BASS_TRACE can't divert the execute path.
        os.environ["BASS_NEVER_TRACE"] = "1"
    nc = _get_nc()
    in_maps = _prep_inputs(**inputs)
    res = run_bass_kernel_spmd(nc, in_maps, list(range(NCORES)), trace=trace)
    full = np.empty((B, T, H), np.float32)
    for i in range(NCORES):
        b, h = i // 2, i % 2
        full[b, h * TQ : (h + 1) * TQ] = res.results[i]["o"]
    return full, res


def kernel(**inputs):
    out, _ = run(inputs, trace=False)
    return out


# revision 41
# speedup vs baseline: 1.6242x; 1.2454x over previous
"""Trainium2 Bass kernel for nn_AttentionHead (B=4, T=4096, D=1024, H=64).

Sharding: 8 cores; core i handles (batch b = i//2, T-half = i%2): computes
attention output for its 2048 queries over all 4096 keys. K/V are computed
per-core over the full 4096 keys (weights tiny/replicated; key order is
permutation-invariant under softmax and is chosen to match DMA arrival).

Design (validated in numpy: max rel err ~5.6e-3 vs the 2e-2 gate):
  - Host pre-transposes x to x^T [D, T] and casts to bf16: no on-device
    transposes of x (the f32r baseline burned 20us of PE + 34us of DVE on
    them), and half the DMA bytes.
  - Projections in bf16 (full PE rate, 1 cyc/row): per 512-row t-group,
    pass [Wk|Wq] (own half) or [Wk|Wv] (other half, and own half again for
    V; the redundant k half is simply not read).  PSUM rows 0:64 = k,
    64:128 = q/v.
  - relu+bias+fp8-quantize on DVE (tensor_scalar add+max), keeping ACT
    exclusively for the exp chain -- the hard bottleneck at ~55us
    payload + ~12us of per-instruction access overhead per core.
  - Scores: fp8e4 DoubleRow matmuls at 0.5 cyc/row.  lhsT k8
    [128, 2, 128] (h zero-padded from 64: walrus requires full-128
    partitions and M=128 for fp8 DR Ldweights), rhs q8 [128, 2, 512]
    (free dim 2x512), out [128 keys, 512 q] f32 = 1 PSUM bank.
  - exp on ACT reads 2-bank score regions [128, 2, 512] in one
    instruction and writes fp8 into per-q-block e8 tiles laid out so
    attn@V consumes them directly ([*, 2kg:2kg+2, :]).
  - attn@V: 256-deep DoubleRow contraction.  lhsT V8 [128, 2, 128]
    (keys kg*256 + j*128 + p; cols 64/65 = ones accumulate the softmax
    denominator, cols 66+ zero-padding), rhs e8 [128, 2, 512]; 16 chained
    matmuls accumulate [128, 512] f32 in one PSUM bank per q-block.
  - Output: copy, PE transpose (f32), reciprocal * scale, DMA out.

Engine budget per core (cost model): ACT 67.7us (bottleneck), PE 45us,
DVE 26us, DMA 29us.  PSUM banks: proj/vt pool 2 + scores 2x2 + o_ps 2 = 8;
q-block 2 steals a proj-pool bank after projections finish.

Scheduling: engines run their streams in order, so emission interleaves
projection groups with attention work.  Key tiles are numbered in DMA
arrival order; all own-half groups go first so every q-block unlocks by
~13us; scores/exp emit per-q-block as soon as tiles land (newest q-block
first during the supply phase -- its backlog uses long-landed tiles);
attn@V chains run as their regions complete, with o_ps banks assigned so
the four chains+output stages drain in parallel at the tail.  PE p-state
and the ACT exp table are pre-warmed.
"""

import os
import numpy as np

B, T, D, H = 4, 4096, 1024, 64
P = 128
TQ = T // 2         # queries per core
GW = 512            # projection t-group width
NG = T // GW        # 8 t-groups (4 own + 4 other)
NC = D // P         # 8 contraction chunks
NKT = T // P        # 32 key tiles of 128
NKG = NKT // 2      # 16 key groups of 256
NQB = TQ // GW      # 4 query blocks of 512
EXPW = 2            # scores region width (PSUM banks per exp instruction)
NCORES = 8

_cache = {}


def _build():
    import concourse.bass as bass
    import concourse.tile as tile
    from concourse import bacc, mybir
    from concourse.masks import make_identity

    f32 = mybir.dt.float32
    bf16 = mybir.dt.bfloat16
    fp8 = mybir.dt.float8e4
    AF = mybir.ActivationFunctionType
    ALU = mybir.AluOpType
    DR = mybir.MatmulPerfMode.DoubleRow

    nc = bacc.Bacc("TRN2", target_bir_lowering=False, debug=False)

    xat = nc.dram_tensor("xat", [D, TQ], bf16, kind="ExternalInput").ap()
    xbt = nc.dram_tensor("xbt", [D, TQ], bf16, kind="ExternalInput").ap()
    wkq = nc.dram_tensor("wkq", [D, P], bf16, kind="ExternalInput").ap()
    wkv = nc.dram_tensor("wkv", [D, P], bf16, kind="ExternalInput").ap()
    bkq = nc.dram_tensor("bkq", [P, 1], f32, kind="ExternalInput").ap()
    bkv = nc.dram_tensor("bkv", [P, 1], f32, kind="ExternalInput").ap()
    out = nc.dram_tensor("o", [TQ, H], f32, kind="ExternalOutput").ap()

    scale = float(1.0 / np.sqrt(H))
    xat_r = xat.rearrange("(c p) t -> p c t", p=P)
    xbt_r = xbt.rearrange("(c p) t -> p c t", p=P)

    with tile.TileContext(nc) as tc:
        with (
            tc.tile_pool(name="const", bufs=1) as constp,
            tc.tile_pool(name="persist", bufs=1) as persist,
            tc.tile_pool(name="xg", bufs=4) as xgp,
            tc.tile_pool(name="v8", bufs=2) as v8p,
            tc.tile_pool(name="e8", bufs=4) as e8p,
            tc.tile_pool(name="osb", bufs=2) as osbp,
            tc.tile_pool(name="of", bufs=2) as ofp,
            tc.tile_pool(name="pv_ps", bufs=2, space="PSUM") as pvp,
            tc.tile_pool(name="sc_ps", bufs=2, space="PSUM") as scop,
            tc.tile_pool(name="o_ps", bufs=2, space="PSUM") as opsp,
        ):
            # ---- constants ----
            wkq_sb = constp.tile([P, NC, P], bf16)
            wkv_sb = constp.tile([P, NC, P], bf16)
            bkq_sb = constp.tile([P, 1], f32)
            bkv_sb = constp.tile([P, 1], f32)
            identf = constp.tile([P, P], f32)
            identb = constp.tile([P, P], bf16)

            # ---- persistent attention operands ----
            # DoubleRow operands must span the full 128 partitions (walrus
            # rejects 64-partition fp8 Ldweights), so the h=64 contraction is
            # zero-padded to (128 partitions x 2 k-tiles): only [0:64, kt, 0, :]
            # is live in k8, the rest is zeroed once at startup.
            k8 = persist.tile([P, NKT, 2, P], fp8, name="k8")
            q8 = persist.tile([P, 2, TQ], fp8, name="q8")
            # V8[p, kg, j, 0:64] = v of key kg*256 + j*128 + p; cols 64/65 =
            # 1.0 so the same matmul accumulates the softmax denominator (row
            # 64).  Padded to the full 128-wide stationary with zeros: walrus
            # only accepts fp8 DoubleRow Ldweights at M=128 with the k-tile
            # pair adjacent in memory (j stride == 128).
            V8 = persist.tile([P, NKG, 2, P], fp8, name="V8")

            # all weight/bias loads are emitted BEFORE any instruction that
            # reads them: a reader emitted ahead of its producer DMA loses the
            # ordering on hardware (first-run garbage reads)
            nc.sync.dma_start(bkq_sb, bkq)
            nc.sync.dma_start(bkv_sb, bkv)
            nc.sync.dma_start(wkq_sb, wkq.rearrange("(c p) m -> p c m", p=P))

            def load_wkv():
                nc.sync.dma_start(wkv_sb, wkv.rearrange("(c p) m -> p c m", p=P))
            make_identity(nc, identf)
            nc.vector.tensor_copy(identb, identf)
            # warm the ACT exp table off the critical path (1.3us load)
            warm = constp.tile([1, 2], fp8)
            nc.scalar.activation(warm[0:1, 0:1], identf[0:1, 0:1], AF.Exp)
            # warm the PE p-state ramp: ~24 back-to-back dummy transposes keep
            # the array busy from ~2.3us so the first projection chain runs at
            # full clock instead of half
            pe_warm = pvp.tile([P, P], bf16, tag="pv", name="pe_warm")
            for _ in range(24):
                nc.tensor.transpose(pe_warm, identb, identb)

            # zero the padded contraction slots and V8 in their NATIVE dtype:
            # bitcast views defeat Tile's alias tracking, so the memsets get
            # no semaphore ordering against the fp8 readers/writers and race
            # on hardware (first-run V8 corruption).  Split across DVE/Pool.
            nc.vector.memset(k8[:, :, 1, :], 0.0)
            nc.gpsimd.memset(k8[H:P, :, 0, :], 0.0)
            nc.gpsimd.memset(q8[:, 1, :], 0.0)
            nc.gpsimd.memset(q8[H:P, 0, :], 0.0)
            nc.gpsimd.memset(V8[:, :, :, 66:P], 0.0)
            nc.gpsimd.memset(V8[:, :, :, 64:66], 1.0)

            # ---- projections ----
            # Key-tile ids are assigned in DMA-ARRIVAL order (own/other halves
            # interleaved), so score regions complete in kt order and the exp
            # chain can start as soon as the first groups land.  Key order is
            # ours to choose: softmax is permutation-invariant over keys as
            # long as K and V use the same order.
            def do_group(g, kt0, first=False, after_dma=None):
                own = g < NG // 2          # own T-half: provides q8 as well
                j = g % (NG // 2)
                src = xat_r if own else xbt_r
                t0 = j * GW
                xg = xgp.tile([P, NC, GW], bf16, tag="xg")
                if first:
                    # first group: 4 smaller transfers so the projection
                    # chain starts as early as possible
                    for c in range(0, NC, 2):
                        nc.sync.dma_start(
                            xg[:, c : c + 2, :], src[:, c : c + 2, t0 : t0 + GW]
                        )
                else:
                    nc.sync.dma_start(
                        xg[:, 0 : NC // 2, :], src[:, 0 : NC // 2, t0 : t0 + GW]
                    )
                    nc.sync.dma_start(
                        xg[:, NC // 2 : NC, :], src[:, NC // 2 : NC, t0 : t0 + GW]
                    )
                if after_dma is not None:
                    after_dma()

                def proj(w_sb):
                    ps = pvp.tile([P, GW], f32, tag="pv", name="proj_ps")
                    for c in range(NC):
                        nc.tensor.matmul(
                            ps, w_sb[:, c, :], xg[:, c, :],
                            start=(c == 0), stop=(c == NC - 1),
                        )
                    return ps

                def relu_k(ps):
                    # first group: split so the first score tiles unblock a
                    # DVE-instruction earlier
                    nkt = 2 if first else GW // P
                    for s in range(0, GW // P, nkt):
                        nc.vector.tensor_scalar(
                            k8[0:H, kt0 + s : kt0 + s + nkt, 0, :],
                            ps[0:H, s * P : (s + nkt) * P].rearrange(
                                "p (a b) -> p a b", b=P
                            ),
                            bkq_sb[0:H, 0:1], 0.0, ALU.add, ALU.max,
                        )

                def relu_v(ps):
                    # bf16 staging: walrus requires stride-2 outputs for fp8
                    # PE transposes, so transpose in bf16 and convert to fp8
                    # during the Pool copy into V8.
                    v8g = v8p.tile([H, GW], bf16, tag="v8")
                    nc.vector.tensor_scalar(
                        v8g, ps[H:P, :], bkv_sb[H:P, 0:1], 0.0, ALU.add, ALU.max,
                    )
                    vt = pvp.tile([P, GW // P, 68], bf16, tag="pv", name="vt_ps")
                    for i in range(GW // P):
                        nc.tensor.transpose(
                            vt[:, i, 0:H],
                            v8g[:, i * P : (i + 1) * P],
                            identb[0:H, 0:H],
                        )
                    for i in range(GW // P):
                        kt = kt0 + i
                        nc.vector.tensor_copy(
                            V8[:, kt // 2, kt % 2, 0:H], vt[:, i, 0:H]
                        )

                if own:
                    ps = proj(wkq_sb)
                    relu_k(ps)
                    nc.vector.tensor_scalar(
                        q8[0:H, 0, t0 : t0 + GW],
                        ps[H:P, :], bkq_sb[H:P, 0:1], 0.0, ALU.add, ALU.max,
                    )
                    relu_v(proj(wkv_sb))
                else:
                    ps = proj(wkv_sb)
                    relu_k(ps)
                    relu_v(ps)

            # ---- interleaved emission ----
            # Engines execute their instruction streams IN ORDER, so attention
            # work must be emitted between projection groups or ACT starves
            # behind the whole projection phase.  Per-qb state machines emit
            # score regions as soon as their key tiles and q-block exist.
            # attn@V + output are strictly qb-serial (single o_ps bank).
            nregions = (NKT + EXPW - 1) // EXPW
            e8t = [None] * NQB
            o_pst = [None] * NQB
            next_r = [0] * NQB
            next_kg = [0] * NQB
            out_done = [False] * NQB

            def emit_regions(qb, kt_done, limit=None):
                q0 = qb * GW
                if e8t[qb] is None:
                    e8t[qb] = e8p.tile([P, NKT, GW], fp8, tag="e8", name=f"e8_{qb}")
                while next_r[qb] < (nregions if limit is None else min(limit, nregions)):
                    r = next_r[qb]
                    ktr = r * EXPW
                    w = min(EXPW, NKT - ktr)
                    if ktr + w > kt_done:
                        break
                    sc = scop.tile([P, EXPW, GW], f32, tag="sc")
                    for i in range(w):
                        nc.tensor.matmul(
                            sc[:, i, :],
                            k8[:, ktr + i, :, :],
                            q8[:, :, q0 : q0 + GW],
                            start=True, stop=True, perf_mode=DR,
                        )
                    nc.scalar.activation(
                        e8t[qb][:, ktr : ktr + w, :], sc[:, 0:w, :], AF.Exp,
                        scale=scale,
                    )
                    next_r[qb] += 1

            def emit_attnv(qb, kt_done):
                # o_ps banks: qb0/qb1 use the two o_ps-pool banks from the
                # start; qb2 takes the proj/vt pool bank once projections are
                # done; qb3 recycles qb0's bank after its output copy.
                if qb == 2 and kt_done < NKT:
                    return
                if qb == 3 and not out_done[0]:
                    return
                while next_kg[qb] < NKG:
                    kg = next_kg[qb]
                    if (2 * kg + 1) // EXPW >= next_r[qb]:
                        break
                    if o_pst[qb] is None:
                        pool, tag = (pvp, "pv") if qb == 2 else (opsp, "ops")
                        o_pst[qb] = pool.tile([P, GW], f32, name=f"o_ps{qb}", tag=tag)
                    nc.tensor.matmul(
                        o_pst[qb],
                        V8[:, kg, :, :],
                        e8t[qb][:, 2 * kg : 2 * kg + 2, :],
                        start=(kg == 0), stop=(kg == NKG - 1), perf_mode=DR,
                    )
                    next_kg[qb] += 1
                if next_kg[qb] == NKG and not out_done[qb]:
                    # the last two q-blocks' output stages land after the exp
                    # chain finishes; route one through the now-idle ACT so
                    # the two final stages drain in parallel instead of
                    # serializing on DVE
                    use_act = qb == NQB - 2
                    q0 = qb * GW
                    o_sb = osbp.tile([H + 2, GW], f32, tag="osb")
                    if use_act:
                        nc.scalar.copy(o_sb, o_pst[qb][0 : H + 2, :])
                    else:
                        nc.vector.tensor_copy(o_sb, o_pst[qb][0 : H + 2, :])
                    oTp, oTt = (pvp, "pv") if qb == 2 else (opsp, "ops")
                    oT = oTp.tile([P, GW // P, 68], f32, tag=oTt, name="oT_ps")
                    oF = ofp.tile([P, GW // P, H], f32, tag="of")
                    for i in range(GW // P):
                        nc.tensor.transpose(
                            oT[:, i, 0 : H + 2],
                            o_sb[:, i * P : (i + 1) * P],
                            identf[0 : H + 2, 0 : H + 2],
                        )
                    recip = osbp.tile([P, GW // P, 1], f32, tag="recip")
                    nc.vector.reciprocal(recip, oT[:, :, H : H + 1])
                    for i in range(GW // P):
                        if use_act:
                            nc.scalar.activation(
                                oF[:, i, :], oT[:, i, 0:H], AF.Copy,
                                scale=recip[:, i, 0:1],
                            )
                        else:
                            nc.vector.tensor_scalar_mul(
                                oF[:, i, :], oT[:, i, 0:H], recip[:, i, 0:1]
                            )
                    outr = out[q0 : q0 + GW, :].rearrange("(i p) h -> p i h", p=P)
                    if qb == NQB - 1:
                        nc.sync.dma_start(outr[:, 0:2, :], oF[:, 0:2, :])
                        nc.sync.dma_start(outr[:, 2:4, :], oF[:, 2:4, :])
                    else:
                        nc.sync.dma_start(outr, oF)
                    out_done[qb] = True

            def emit_ready(kt_done, qb_unlocked, newest_first=False):
                order = range(qb_unlocked)
                if newest_first:
                    # supply phase: the newly-unlocked q-block's backlog uses
                    # long-landed key tiles -- emit it ahead of older
                    # q-blocks' regions for the just-landed tiles so the ACT
                    # queue stays in readiness order
                    order = reversed(list(order))
                for qb in order:
                    emit_regions(qb, kt_done)
                for qb in range(qb_unlocked):
                    emit_attnv(qb, kt_done)

            # own groups first: every q-block is unlocked by ~13us and the
            # exp chain never starves (region supply outpaces ACT ~3x)
            for pos in range(NG):
                do_group(pos, kt0=4 * pos, first=(pos == 0),
                         after_dma=load_wkv if pos == 0 else None)
                emit_ready(4 * (pos + 1), min(pos + 1, NQB),
                           newest_first=(pos < NG - 1))
            while not all(out_done):
                emit_ready(NKT, NQB)

    nc.compile()
    return nc


def _get_nc():
    if "nc" not in _cache:
        _cache["nc"] = _build()
    return _cache["nc"]


def _prep_inputs(x, Wk, bk, Wq, bq, Wv, bv):
    import ml_dtypes

    bf16 = ml_dtypes.bfloat16
    x = np.asarray(x, np.float32)
    wkq = np.concatenate(
        [np.asarray(Wk, np.float32).T, np.asarray(Wq, np.float32).T], axis=1
    ).astype(bf16)
    wkv = np.concatenate(
        [np.asarray(Wk, np.float32).T, np.asarray(Wv, np.float32).T], axis=1
    ).astype(bf16)
    bkq = np.concatenate(
        [np.asarray(bk, np.float32), np.asarray(bq, np.float32)]
    ).reshape(P, 1)
    bkv = np.concatenate(
        [np.asarray(bk, np.float32), np.asarray(bv, np.float32)]
    ).reshape(P, 1)
    in_maps = []
    for i in range(NCORES):
        b, h = i // 2, i % 2
        xat = np.ascontiguousarray(x[b, h * TQ : (h + 1) * TQ].T).astype(bf16)
        xbt = np.ascontiguousarray(x[b, (1 - h) * TQ : (2 - h) * TQ].T).astype(bf16)
        in_maps.append(dict(xat=xat, xbt=xbt, wkq=wkq, wkv=wkv, bkq=bkq, bkv=bkv))
    return in_maps


def run(inputs, trace=False):
    from concourse.bass_utils import run_bass_kernel_spmd

    if not trace:
        # NTFF profiling is unavailable in this environment; make sure an
        # ambient BASS_TRACE can't divert the execute path.
        os.environ["BASS_NEVER_TRACE"] = "1"
    nc = _get_nc()
    in_maps = _prep_inputs(**inputs)
    res = run_bass_kernel_spmd(nc, in_maps, list(range(NCORES)), trace=trace)
    full = np.empty((B, T, H), np.float32)
    for i in range(NCORES):
        b, h = i // 2, i % 2
        full[b, h * TQ : (h + 1) * TQ] = res.results[i]["o"]
    return full, res


def kernel(**inputs):
    out, _ = run(inputs, trace=False)
    return out


# revision 47
# speedup vs baseline: 1.6765x; 1.0322x over previous
"""Trainium2 Bass kernel for nn_AttentionHead (B=4, T=4096, D=1024, H=64).

Sharding: 8 cores; core i handles (batch b = i//2, T-half = i%2): computes
attention output for its 2048 queries over all 4096 keys. K/V are computed
per-core over the full 4096 keys (weights tiny/replicated; key order is
permutation-invariant under softmax and is chosen to match DMA arrival).

Design (validated in numpy: max rel err ~5.6e-3 vs the 2e-2 gate):
  - Host pre-transposes x to x^T [D, T] and casts to bf16: no on-device
    transposes of x (the f32r baseline burned 20us of PE + 34us of DVE on
    them), and half the DMA bytes.
  - Projections in bf16 (full PE rate, 1 cyc/row): per 512-row t-group,
    pass [Wk|Wq] (own half) or [Wk|Wv] (other half, and own half again for
    V; the redundant k half is simply not read).  PSUM rows 0:64 = k,
    64:128 = q/v.
  - relu+bias+fp8-quantize on DVE (tensor_scalar add+max), keeping ACT
    exclusively for the exp chain -- the hard bottleneck at ~55us
    payload + ~12us of per-instruction access overhead per core.
  - Scores: fp8e4 DoubleRow matmuls at 0.5 cyc/row.  lhsT k8
    [128, 2, 128] (h zero-padded from 64: walrus requires full-128
    partitions and M=128 for fp8 DR Ldweights), rhs q8 [128, 2, 512]
    (free dim 2x512), out [128 keys, 512 q] f32 = 1 PSUM bank.
  - exp on ACT reads 2-bank score regions [128, 2, 512] in one
    instruction and writes fp8 into per-q-block e8 tiles laid out so
    attn@V consumes them directly ([*, 2kg:2kg+2, :]).
  - attn@V: 256-deep DoubleRow contraction.  lhsT V8 [128, 2, 128]
    (keys kg*256 + j*128 + p; cols 64/65 = ones accumulate the softmax
    denominator, cols 66+ zero-padding), rhs e8 [128, 2, 512]; 16 chained
    matmuls accumulate [128, 512] f32 in one PSUM bank per q-block.
  - Output: copy, PE transpose (f32), reciprocal * scale, DMA out.

Engine budget per core (cost model): ACT 67.7us (bottleneck), PE 45us,
DVE 26us, DMA 29us.  PSUM banks: proj/vt pool 2 + scores 2x2 + o_ps 2 = 8;
q-block 2 steals a proj-pool bank after projections finish.

Scheduling: engines run their streams in order, so emission interleaves
projection groups with attention work.  Key tiles are numbered in DMA
arrival order; all own-half groups go first so every q-block unlocks by
~13us; scores/exp emit per-q-block as soon as tiles land (newest q-block
first during the supply phase -- its backlog uses long-landed tiles);
attn@V chains run as their regions complete, with o_ps banks assigned so
the four chains+output stages drain in parallel at the tail.  PE p-state
and the ACT exp table are pre-warmed.
"""

import os
import numpy as np

B, T, D, H = 4, 4096, 1024, 64
P = 128
TQ = T // 2         # queries per core
GW = 512            # projection t-group width
NG = T // GW        # 8 t-groups (4 own + 4 other)
NC = D // P         # 8 contraction chunks
NKT = T // P        # 32 key tiles of 128
NKG = NKT // 2      # 16 key groups of 256
NQB = TQ // GW      # 4 query blocks of 512
EXPW = 2            # scores region width (PSUM banks per exp instruction)
NCORES = 8

_cache = {}


def _build():
    import concourse.bass as bass
    import concourse.tile as tile
    from concourse import bacc, mybir
    from concourse.masks import make_identity

    f32 = mybir.dt.float32
    bf16 = mybir.dt.bfloat16
    fp8 = mybir.dt.float8e4
    AF = mybir.ActivationFunctionType
    ALU = mybir.AluOpType
    DR = mybir.MatmulPerfMode.DoubleRow

    nc = bacc.Bacc("TRN2", target_bir_lowering=False, debug=False)

    xat = nc.dram_tensor("xat", [D, TQ], bf16, kind="ExternalInput").ap()
    xbt = nc.dram_tensor("xbt", [D, TQ], bf16, kind="ExternalInput").ap()
    wkq = nc.dram_tensor("wkq", [D, P], bf16, kind="ExternalInput").ap()
    wkv = nc.dram_tensor("wkv", [D, P], bf16, kind="ExternalInput").ap()
    bkq = nc.dram_tensor("bkq", [P, 1], f32, kind="ExternalInput").ap()
    bkv = nc.dram_tensor("bkv", [P, 1], f32, kind="ExternalInput").ap()
    out = nc.dram_tensor("o", [TQ, H], f32, kind="ExternalOutput").ap()

    scale = float(1.0 / np.sqrt(H))
    xat_r = xat.rearrange("(c p) t -> p c t", p=P)
    xbt_r = xbt.rearrange("(c p) t -> p c t", p=P)

    with tile.TileContext(nc) as tc:
        with (
            tc.tile_pool(name="const", bufs=1) as constp,
            tc.tile_pool(name="persist", bufs=1) as persist,
            tc.tile_pool(name="xg", bufs=4) as xgp,
            tc.tile_pool(name="v8", bufs=2) as v8p,
            tc.tile_pool(name="e8", bufs=4) as e8p,
            tc.tile_pool(name="osb", bufs=2) as osbp,
            tc.tile_pool(name="of", bufs=2) as ofp,
            tc.tile_pool(name="pv_ps", bufs=2, space="PSUM") as pvp,
            tc.tile_pool(name="sc_ps", bufs=2, space="PSUM") as scop,
            tc.tile_pool(name="o_ps", bufs=2, space="PSUM") as opsp,
        ):
            # ---- constants ----
            wkq_sb = constp.tile([P, NC, P], bf16)
            wkv_sb = constp.tile([P, NC, P], bf16)
            bkq_sb = constp.tile([P, 1], f32)
            bkv_sb = constp.tile([P, 1], f32)
            identf = constp.tile([P, P], f32)
            identb = constp.tile([P, P], bf16)

            # ---- persistent attention operands ----
            # DoubleRow operands must span the full 128 partitions (walrus
            # rejects 64-partition fp8 Ldweights), so the h=64 contraction is
            # zero-padded to (128 partitions x 2 k-tiles): only [0:64, kt, 0, :]
            # is live in k8, the rest is zeroed once at startup.
            k8 = persist.tile([P, NKT, 2, P], fp8, name="k8")
            q8 = persist.tile([P, 2, TQ], fp8, name="q8")
            # V8[p, kg, j, 0:64] = v of key kg*256 + j*128 + p; cols 64/65 =
            # 1.0 so the same matmul accumulates the softmax denominator (row
            # 64).  Padded to the full 128-wide stationary with zeros: walrus
            # only accepts fp8 DoubleRow Ldweights at M=128 with the k-tile
            # pair adjacent in memory (j stride == 128).
            V8 = persist.tile([P, NKG, 2, P], fp8, name="V8")

            # all weight/bias loads are emitted BEFORE any instruction that
            # reads them: a reader emitted ahead of its producer DMA loses the
            # ordering on hardware (first-run garbage reads)
            nc.sync.dma_start(bkq_sb, bkq)
            nc.sync.dma_start(bkv_sb, bkv)
            wkq_r = wkq.rearrange("(c p) m -> p c m", p=P)
            nc.sync.dma_start(wkq_sb[:, 0 : NC // 2, :], wkq_r[:, 0 : NC // 2, :])

            def load_wkq_b():
                nc.sync.dma_start(wkq_sb[:, NC // 2 : NC, :], wkq_r[:, NC // 2 : NC, :])

            def load_wkv():
                nc.sync.dma_start(wkv_sb, wkv.rearrange("(c p) m -> p c m", p=P))
            make_identity(nc, identf)
            nc.vector.tensor_copy(identb, identf)
            # warm the ACT exp table off the critical path (1.3us load)
            warm = constp.tile([1, 2], fp8)
            nc.scalar.activation(warm[0:1, 0:1], identf[0:1, 0:1], AF.Exp)
            # zero the padded contraction slots and V8 in their NATIVE dtype:
            # bitcast views defeat Tile's alias tracking, so the memsets get
            # no semaphore ordering against the fp8 readers/writers and race
            # on hardware (first-run V8 corruption).  Split across DVE/Pool.
            # the slices gating the FIRST score regions (key tiles 0:4 and
            # query block 0) are zeroed first so the exp chain is not stuck
            # behind ~7us of serial pad clearing
            nc.vector.memset(k8[:, 0:4, 1, :], 0.0)
            nc.gpsimd.memset(k8[H:P, 0:4, 0, :], 0.0)
            nc.gpsimd.memset(q8[:, 1, 0:GW], 0.0)
            nc.gpsimd.memset(q8[H:P, 0, 0:GW], 0.0)
            nc.vector.memset(k8[:, 4:NKT, 1, :], 0.0)
            nc.gpsimd.memset(k8[H:P, 4:NKT, 0, :], 0.0)
            nc.gpsimd.memset(q8[:, 1, GW:TQ], 0.0)
            nc.gpsimd.memset(q8[H:P, 0, GW:TQ], 0.0)
            nc.gpsimd.memset(V8[:, :, :, 66:P], 0.0)
            nc.gpsimd.memset(V8[:, :, :, 64:66], 1.0)

            # ---- projections ----
            # Key-tile ids are assigned in DMA-ARRIVAL order (own/other halves
            # interleaved), so score regions complete in kt order and the exp
            # chain can start as soon as the first groups land.  Key order is
            # ours to choose: softmax is permutation-invariant over keys as
            # long as K and V use the same order.
            def do_group(g, kt0, first=False, after_dma=None):
                own = g < NG // 2          # own T-half: provides q8 as well
                j = g % (NG // 2)
                src = xat_r if own else xbt_r
                t0 = j * GW
                xg = xgp.tile([P, NC, GW], bf16, tag="xg")
                if first:
                    # first group: 4 smaller transfers, with the second wkq
                    # half slotted between them, so the projection chain
                    # tails the DMA queue as tightly as possible
                    for c in range(0, NC, 2):
                        nc.sync.dma_start(
                            xg[:, c : c + 2, :], src[:, c : c + 2, t0 : t0 + GW]
                        )
                        if c == 2:
                            load_wkq_b()
                else:
                    nc.sync.dma_start(
                        xg[:, 0 : NC // 2, :], src[:, 0 : NC // 2, t0 : t0 + GW]
                    )
                    nc.sync.dma_start(
                        xg[:, NC // 2 : NC, :], src[:, NC // 2 : NC, t0 : t0 + GW]
                    )
                if after_dma is not None:
                    after_dma()

                def proj(w_sb):
                    ps = pvp.tile([P, GW], f32, tag="pv", name="proj_ps")
                    for c in range(NC):
                        nc.tensor.matmul(
                            ps, w_sb[:, c, :], xg[:, c, :],
                            start=(c == 0), stop=(c == NC - 1),
                        )
                    return ps

                def relu_k(ps):
                    # first group: split so the first score tiles unblock a
                    # DVE-instruction earlier
                    nkt = 2 if first else GW // P
                    for s in range(0, GW // P, nkt):
                        nc.vector.tensor_scalar(
                            k8[0:H, kt0 + s : kt0 + s + nkt, 0, :],
                            ps[0:H, s * P : (s + nkt) * P].rearrange(
                                "p (a b) -> p a b", b=P
                            ),
                            bkq_sb[0:H, 0:1], 0.0, ALU.add, ALU.max,
                        )

                def relu_v(ps):
                    # bf16 staging: walrus requires stride-2 outputs for fp8
                    # PE transposes, so transpose in bf16 and convert to fp8
                    # during the Pool copy into V8.
                    v8g = v8p.tile([H, GW], bf16, tag="v8")
                    nc.vector.tensor_scalar(
                        v8g, ps[H:P, :], bkv_sb[H:P, 0:1], 0.0, ALU.add, ALU.max,
                    )
                    vt = pvp.tile([P, GW // P, 68], bf16, tag="pv", name="vt_ps")
                    for i in range(GW // P):
                        nc.tensor.transpose(
                            vt[:, i, 0:H],
                            v8g[:, i * P : (i + 1) * P],
                            identb[0:H, 0:H],
                        )
                    for i in range(GW // P):
                        kt = kt0 + i
                        nc.vector.tensor_copy(
                            V8[:, kt // 2, kt % 2, 0:H], vt[:, i, 0:H]
                        )

                if own:
                    ps = proj(wkq_sb)
                    relu_k(ps)
                    nc.vector.tensor_scalar(
                        q8[0:H, 0, t0 : t0 + GW],
                        ps[H:P, :], bkq_sb[H:P, 0:1], 0.0, ALU.add, ALU.max,
                    )
                    relu_v(proj(wkv_sb))
                else:
                    ps = proj(wkv_sb)
                    relu_k(ps)
                    relu_v(ps)

            # ---- interleaved emission ----
            # Engines execute their instruction streams IN ORDER, so attention
            # work must be emitted between projection groups or ACT starves
            # behind the whole projection phase.  Per-qb state machines emit
            # score regions as soon as their key tiles and q-block exist.
            # attn@V + output are strictly qb-serial (single o_ps bank).
            nregions = (NKT + EXPW - 1) // EXPW
            e8t = [None] * NQB
            o_pst = [None] * NQB
            next_r = [0] * NQB
            next_kg = [0] * NQB
            out_done = [False] * NQB

            def emit_regions(qb, kt_done, limit=None):
                q0 = qb * GW
                if e8t[qb] is None:
                    e8t[qb] = e8p.tile([P, NKT, GW], fp8, tag="e8", name=f"e8_{qb}")
                while next_r[qb] < (nregions if limit is None else min(limit, nregions)):
                    r = next_r[qb]
                    ktr = r * EXPW
                    w = min(EXPW, NKT - ktr)
                    if ktr + w > kt_done:
                        break
                    sc = scop.tile([P, EXPW, GW], f32, tag="sc")
                    for i in range(w):
                        nc.tensor.matmul(
                            sc[:, i, :],
                            k8[:, ktr + i, :, :],
                            q8[:, :, q0 : q0 + GW],
                            start=True, stop=True, perf_mode=DR,
                        )
                    nc.scalar.activation(
                        e8t[qb][:, ktr : ktr + w, :], sc[:, 0:w, :], AF.Exp,
                        scale=scale,
                    )
                    next_r[qb] += 1

            def emit_attnv(qb, kt_done):
                # o_ps banks: qb0/qb1 use the two o_ps-pool banks from the
                # start; qb2 takes the proj/vt pool bank once projections are
                # done; qb3 recycles qb0's bank after its output copy.
                if qb == 2 and kt_done < NKT:
                    return
                if qb == 3 and not out_done[0]:
                    return
                while next_kg[qb] < NKG:
                    kg = next_kg[qb]
                    if (2 * kg + 1) // EXPW >= next_r[qb]:
                        break
                    if o_pst[qb] is None:
                        pool, tag = (pvp, "pv") if qb == 2 else (opsp, "ops")
                        o_pst[qb] = pool.tile([P, GW], f32, name=f"o_ps{qb}", tag=tag)
                    nc.tensor.matmul(
                        o_pst[qb],
                        V8[:, kg, :, :],
                        e8t[qb][:, 2 * kg : 2 * kg + 2, :],
                        start=(kg == 0), stop=(kg == NKG - 1), perf_mode=DR,
                    )
                    next_kg[qb] += 1
                if next_kg[qb] == NKG and not out_done[qb]:
                    # the last two q-blocks' output stages land after the exp
                    # chain finishes; route one through the now-idle ACT so
                    # the two final stages drain in parallel instead of
                    # serializing on DVE
                    use_act = qb == NQB - 2
                    q0 = qb * GW
                    o_sb = osbp.tile([H + 2, GW], f32, tag="osb")
                    if use_act:
                        nc.scalar.copy(o_sb, o_pst[qb][0 : H + 2, :])
                    else:
                        nc.vector.tensor_copy(o_sb, o_pst[qb][0 : H + 2, :])
                    oTp, oTt = (pvp, "pv") if qb == 2 else (opsp, "ops")
                    oT = oTp.tile([P, GW // P, 68], f32, tag=oTt, name="oT_ps")
                    oF = ofp.tile([P, GW // P, H], f32, tag="of")
                    for i in range(GW // P):
                        nc.tensor.transpose(
                            oT[:, i, 0 : H + 2],
                            o_sb[:, i * P : (i + 1) * P],
                            identf[0 : H + 2, 0 : H + 2],
                        )
                    recip = osbp.tile([P, GW // P, 1], f32, tag="recip")
                    nc.vector.reciprocal(recip, oT[:, :, H : H + 1])
                    for i in range(GW // P):
                        if use_act:
                            nc.scalar.activation(
                                oF[:, i, :], oT[:, i, 0:H], AF.Copy,
                                scale=recip[:, i, 0:1],
                            )
                        else:
                            nc.vector.tensor_scalar_mul(
                                oF[:, i, :], oT[:, i, 0:H], recip[:, i, 0:1]
                            )
                    outr = out[q0 : q0 + GW, :].rearrange("(i p) h -> p i h", p=P)
                    if qb == NQB - 1:
                        nc.sync.dma_start(outr[:, 0:2, :], oF[:, 0:2, :])
                        nc.sync.dma_start(outr[:, 2:4, :], oF[:, 2:4, :])
                    else:
                        nc.sync.dma_start(outr, oF)
                    out_done[qb] = True

            def emit_ready(kt_done, qb_unlocked, newest_first=False):
                order = range(qb_unlocked)
                if newest_first:
                    # supply phase: the newly-unlocked q-block's backlog uses
                    # long-landed key tiles -- emit it ahead of older
                    # q-blocks' regions for the just-landed tiles so the ACT
                    # queue stays in readiness order
                    order = reversed(list(order))
                for qb in order:
                    emit_regions(qb, kt_done)
                for qb in range(qb_unlocked):
                    emit_attnv(qb, kt_done)

            # own groups first: every q-block is unlocked by ~13us and the
            # exp chain never starves (region supply outpaces ACT ~3x)
            for pos in range(NG):
                do_group(pos, kt0=4 * pos, first=(pos == 0),
                         after_dma=load_wkv if pos == 0 else None)
                emit_ready(4 * (pos + 1), min(pos + 1, NQB),
                           newest_first=(pos < NG - 1))
            while not all(out_done):
                emit_ready(NKT, NQB)

    nc.compile()
    return nc


def _get_nc():
    if "nc" not in _cache:
        _cache["nc"] = _build()
    return _cache["nc"]


def _prep_inputs(x, Wk, bk, Wq, bq, Wv, bv):
    import ml_dtypes

    bf16 = ml_dtypes.bfloat16
    x = np.asarray(x, np.float32)
    wkq = np.concatenate(
        [np.asarray(Wk, np.float32).T, np.asarray(Wq, np.float32).T], axis=1
    ).astype(bf16)
    wkv = np.concatenate(
        [np.asarray(Wk, np.float32).T, np.asarray(Wv, np.float32).T], axis=1
    ).astype(bf16)
    bkq = np.concatenate(
        [np.asarray(bk, np.float32), np.asarray(bq, np.float32)]
    ).reshape(P, 1)
    bkv = np.concatenate(
        [np.asarray(bk, np.float32), np.asarray(bv, np.float32)]
    ).reshape(P, 1)
    in_maps = []
    for i in range(NCORES):
        b, h = i // 2, i % 2
        xat = np.ascontiguousarray(x[b, h * TQ : (h + 1) * TQ].T).astype(bf16)
        xbt = np.ascontiguousarray(x[b, (1 - h) * TQ : (2 - h) * TQ].T).astype(bf16)
        in_maps.append(dict(xat=xat, xbt=xbt, wkq=wkq, wkv=wkv, bkq=bkq, bkv=bkv))
    return in_maps


def run(inputs, trace=False):
    from concourse.bass_utils import run_bass_kernel_spmd

    if not trace:
        # NTFF profiling is unavailable in this environment; make sure an
        # ambient BASS_TRACE can't divert the execute path.
        os.environ["BASS_NEVER_TRACE"] = "1"
    nc = _get_nc()
    in_maps = _prep_inputs(**inputs)
    res = run_bass_kernel_spmd(nc, in_maps, list(range(NCORES)), trace=trace)
    full = np.empty((B, T, H), np.float32)
    for i in range(NCORES):
        b, h = i // 2, i % 2
        full[b, h * TQ : (h + 1) * TQ] = res.results[i]["o"]
    return full, res


def kernel(**inputs):
    out, _ = run(inputs, trace=False)
    return out
